# revision 18
# baseline (speedup 1.0000x reference)
"""Trainium2 Bass kernel for per-token head-attention transformer block.

Reference computation (N=16, T=4096, D=1024, H=16, hd=64):
    qkv = x @ w_qkv + b_qkv                       (N,T,3D)
    q,k,v = split(qkv)  each (N,T,H,hd)
    S = einsum('nthd,ntgd->nthg', q*hd^-0.5, k)   per-token 16x16 over heads
    P = softmax(S, -1)
    o = einsum('nthg,ntgd->nthd', P, v)
    out = o.transpose(0,2,1,3).reshape(N,T,D) @ w_proj + b_proj

Mapping: data-parallel over batch N across 8 cores (2 batch elements each).
Per core the kernel is vector-engine-bound (the per-token 16x16 attention is
~32K elementwise ops/token/phase); tuning knobs:
  - QK_GROUPS / AV_GROUPS: (h0, nh, engine) head-group split between DVE
    ('v', bf16 tensor_tensor at 2x mode) and GPSIMD ('p'); balance by
    max(DVE_time, pool_penalty * Pool_time) where pool_penalty is 1.0 per
    the CoreSim cost model but ~2.5 per measured Q7 two-input throughput.
  - pool bufs: xp/xtp depth pipelines the phase-2 (projection) chunks;
    psA/psT split the 8 PSUM banks between matmul accumulation and
    transpose-pair evacuation.
Design points:
  - all matmuls in bf16 (same PE rate as f32r, half the SBUF, FWL weight loads)
  - attention math restructured: one fused product per head-group, then an
    in-place halving add-tree (writes back into the product tile's lower
    half); the last level writes S / O directly
  - no max-subtraction in softmax (scores are O(5) for this data; exp in f32)
  - a tunable slice of the head-groups runs on GPSIMD (tensor_tensor never
    contends with DVE ports), overlapping the two vector engines
  - v is written straight into its (d,g)-interleaved tile during the
    PSUM-evacuation of the v transposes (no separate re-copy, no v section
    in the token-major buffer)
  - PSUM evacuations all on ACT; DVE does only attention math
  - attention output spilled to DRAM in bf16 (halves spill traffic)
"""

import sys

sys.path.insert(0, "/opt/trn_rl_repo")

from contextlib import ExitStack

import numpy as np

import concourse.bass as bass
import concourse.tile as tile
from concourse import mybir
from concourse.bass_utils import run_bass_kernel_spmd
from concourse.masks import make_identity

N, T, D = 16, 4096, 1024
H, HD = 16, 64
NCORES = 8
NB = N // NCORES  # batch elements per core
SCALE = float(HD) ** -0.5

F32 = mybir.dt.float32
BF16 = mybir.dt.bfloat16
ATT_DT = BF16

CH = 256          # token chunk (matmul moving dim)
NT = CH // 128    # token tiles per chunk
KD = D // 128     # contraction chunks (8)
JQ = 3 * D // 128  # qkv output feature chunks (24)
JP = D // 128     # proj output feature chunks (8)
NCH = T // CH     # chunks per batch element (16)

# head-group split across the two vector engines: list of (h0, nh, engine)
# engine: 'v' = DVE, 'p' = GPSIMD
# GPSIMD (Pool) shares its SBUF port with DVE: HW-measured, independent Pool
# tensor_tensor ops overlap only ~15% with a busy DVE stream, so offloading
# head-groups to Pool is a net loss. All attention math runs on DVE.
QK_GROUPS = [(0, 16, "v")]
AV_GROUPS = [(0, 16, "v")]

Ident = mybir.ActivationFunctionType.Identity
Exp = mybir.ActivationFunctionType.Exp
ALU = mybir.AluOpType
AX = mybir.AxisListType


def _ap(sl, dims):
    """Custom free-dim access pattern on a sliced tile: keep partition dim +
    offset of `sl`, replace free dims with [step, num] list `dims`."""
    return bass.AP(tensor=sl.tensor, offset=sl.offset, ap=[sl.ap[0]] + dims)


def build_kernel():
    nc = bass.Bass()
    x = nc.dram_tensor("x", [NB * T, D], F32, kind="ExternalInput")
    wqkv = nc.dram_tensor("w_qkv", [D, 3 * D], F32, kind="ExternalInput")
    bqkv = nc.dram_tensor("b_qkv", [3 * D], F32, kind="ExternalInput")
    wproj = nc.dram_tensor("w_proj", [D, D], F32, kind="ExternalInput")
    bproj = nc.dram_tensor("b_proj", [D], F32, kind="ExternalInput")
    y = nc.dram_tensor("y", [NB * T, D], F32, kind="ExternalOutput")

    with ExitStack() as ctx:
        tc = ctx.enter_context(tile.TileContext(nc))
        singles = ctx.enter_context(tc.tile_pool(name="singles", bufs=1))
        xp = ctx.enter_context(tc.tile_pool(name="xp", bufs=2))
        xtp = ctx.enter_context(tc.tile_pool(name="xtp", bufs=2))
        ytp = ctx.enter_context(tc.tile_pool(name="ytp", bufs=1))
        qkvp = ctx.enter_context(tc.tile_pool(name="qkvp", bufs=1))
        tokp = ctx.enter_context(tc.tile_pool(name="tokp", bufs=1))
        att = ctx.enter_context(tc.tile_pool(name="att", bufs=1))
        outp = ctx.enter_context(tc.tile_pool(name="outp", bufs=1))
        prodp = ctx.enter_context(tc.tile_pool(name="prodp", bufs=1))
        vtp = ctx.enter_context(tc.tile_pool(name="vtp", bufs=2))
        psA = ctx.enter_context(tc.tile_pool(name="psA", bufs=3, space="PSUM"))
        psT = ctx.enter_context(tc.tile_pool(name="psT", bufs=5, space="PSUM"))
        dram = ctx.enter_context(tc.tile_pool(name="dram", bufs=1, space="DRAM"))

        ident = singles.tile([128, 128], F32)
        make_identity(nc, ident)
        ident_b = singles.tile([128, 128], ATT_DT)
        make_identity(nc, ident_b)

        # resident weights in bf16, (in,out) layout chunked over contraction dim
        wq_s = singles.tile([128, KD, 3 * D], BF16)
        wq_src = wqkv.rearrange("(k p) j -> p k j", p=128)
        for k in range(KD):
            nc.gpsimd.dma_start(out=wq_s[:, k, :], in_=wq_src[:, k, :])
        wp_s = singles.tile([128, KD, D], BF16)
        wp_src = wproj.rearrange("(k p) j -> p k j", p=128)
        for k in range(KD):
            nc.gpsimd.dma_start(out=wp_s[:, k, :], in_=wp_src[:, k, :])
        # biases, one merged tile: cols [0,JQ) = b_qkv, [JQ,JQ+JP) = b_proj,
        # [JQ+JP, ...) = SCALE * b_q
        bias = singles.tile([128, JQ + JP + JQ // 3], F32)
        nc.gpsimd.dma_start(
            out=bias[:, 0:JQ], in_=bqkv.rearrange("(j p) -> p j", p=128)
        )
        nc.gpsimd.dma_start(
            out=bias[:, JQ : JQ + JP], in_=bproj.rearrange("(j p) -> p j", p=128)
        )
        nc.scalar.mul(bias[:, JQ + JP :], bias[:, 0 : JQ // 3], SCALE)

        # head-major attention-output spill (bf16): flat layout
        # h*(T*HD) + t*HD + d, viewed by phase 2 as a row-major (T, D)
        # matrix per batch element.
        aspill = dram.tile([NB, T, D], ATT_DT)

        def halving_tree(prod, nblk, inner, out_final):
            """Sum the innermost dim of prod [128, ..., inner] (nblk blocks
            of `inner`) by repeated in-place halving adds (out aliases the
            low half of each block); the final level writes out_final.
            APs are emitted pre-collapsed to 2 free dims (ISA limit is 3)."""
            base = prod[:, 0, 0, 0, 0]
            m = inner
            while m > 1:
                half = m // 2
                if half == 1:
                    dst = out_final
                else:
                    dst = _ap(base, [[inner, nblk], [1, half]])
                nc.vector.tensor_tensor(
                    out=dst,
                    in0=_ap(base, [[inner, nblk], [1, half]]),
                    in1=_ap(prod[:, 0, 0, 0, half], [[inner, nblk], [1, half]]),
                    op=ALU.add,
                )
                m = half

        def p1_chunk(n, c):
            t0 = c * CH
            xT = xtp.tile([128, KD, CH], BF16, tag="xT")
            for tt in range(NT):
                xt = xp.tile([128, D], F32, tag="x")
                r0 = n * T + t0 + tt * 128
                nc.sync.dma_start(out=xt, in_=x[r0 : r0 + 128, :])
                for k in range(0, KD, 2):
                    pt = psT.tile([128, 2, 128], F32, tag="tp")
                    for kk in range(2):
                        nc.tensor.transpose(
                            pt[:, kk, :],
                            xt[:, (k + kk) * 128 : (k + kk + 1) * 128],
                            ident,
                        )
                    nc.scalar.copy(
                        out=_ap(xT[:, k, tt * 128], [[CH, 2], [1, 128]]),
                        in_=pt,
                    )

            qkvT = qkvp.tile([128, JQ, CH], ATT_DT, tag="qkvT")
            for j in range(JQ):
                pm = psA.tile([128, CH], F32, tag="mm")
                for k in range(KD):
                    nc.tensor.matmul(
                        pm,
                        wq_s[:, k, j * 128 : (j + 1) * 128],
                        xT[:, k, :],
                        start=(k == 0),
                        stop=(k == KD - 1),
                    )
                if j < JQ // 3:  # q: fold in attention scale
                    nc.scalar.activation(
                        out=qkvT[:, j, :], in_=pm, func=Ident,
                        bias=bias[:, JQ + JP + j : JQ + JP + j + 1], scale=SCALE,
                    )
                else:
                    nc.scalar.activation(
                        out=qkvT[:, j, :], in_=pm, func=Ident,
                        bias=bias[:, j : j + 1], scale=1.0,
                    )

            # token-major marshalling for BOTH tiles first, so the ACT queue
            # never has an exp (which waits on DVE) ahead of evac work;
            # both 128-token tiles live in one [128, NT, ...] tile so the
            # attention math runs as one fused op per step (DVE per-op
            # overhead on HW is ~0.4us, so fewer+bigger ops win)
            tokc = tokp.tile([128, NT, 2 * D], ATT_DT, tag="tok")
            vtc = vtp.tile([128, NT, HD, H], ATT_DT, tag="vt")
            for tt in range(NT):
                # feature-major -> token-major for the per-token attention;
                # q,k go to `tokc`, v goes straight into the (d,g) layout
                for j in range(0, JQ, 2):
                    pt = psT.tile([128, 2, 128], ATT_DT, tag="tp")
                    for jj in range(2):
                        nc.tensor.transpose(
                            pt[:, jj, :],
                            qkvT[:, j + jj, tt * 128 : (tt + 1) * 128],
                            ident_b,
                        )
                    if j < 2 * JQ // 3:
                        nc.scalar.copy(
                            out=tokc[:, tt, j * 128 : (j + 2) * 128], in_=pt
                        )
                    else:
                        jv = j - 2 * JQ // 3
                        nc.scalar.copy(
                            out=_ap(
                                vtc[:, tt, 0, 2 * jv], [[2, 2], [1, 2], [H, HD]]
                            ),
                            in_=_ap(
                                pt[:, 0, 0], [[128, 2], [HD, 2], [1, HD]]
                            ),
                        )

            # scores S[t,h,g] = sum_d q[t,h,d] k[t,g,d] (q pre-scaled);
            # products per tile (the broadcast APs don't fit the ISA's 3
            # free dims when fused), tree levels fused over both tiles
            # (contiguous APs collapse)
            S = att.tile([128, NT, H, H], F32, tag="S")
            prod = prodp.tile([128, NT, H, H, HD], ATT_DT, tag="prod")
            for tt in range(NT):
                nc.vector.tensor_tensor(
                    out=prod[:, tt],
                    in0=_ap(tokc[:, tt, 0], [[HD, H], [0, H], [1, HD]]),
                    in1=_ap(tokc[:, tt, D], [[0, H], [HD, H], [1, HD]]),
                    op=ALU.mult,
                )
            halving_tree(
                prod, NT * H * H, HD,
                _ap(S[:, 0, 0, 0], [[1, NT * H * H], [1, 1]]),
            )

            # softmax over g without max-subtraction (|S| is O(5)); exp on
            # ACT per tile, side-ops on DVE (Pool contends with DVE ports)
            Sx = att.tile([128, NT, H, H], ATT_DT, tag="Sx")
            stats = att.tile([128, NT, 2, H], F32, tag="stats")
            P = att.tile([128, NT, H, H], ATT_DT, tag="P")
            for tt in range(NT):
                nc.scalar.activation(out=Sx[:, tt], in_=S[:, tt], func=Exp)
                nc.vector.tensor_reduce(
                    out=stats[:, tt, 0, :], in_=Sx[:, tt], axis=AX.X, op=ALU.add
                )
                nc.vector.reciprocal(stats[:, tt, 1, :], stats[:, tt, 0, :])
                nc.vector.tensor_tensor(
                    out=P[:, tt], in0=Sx[:, tt],
                    in1=_ap(stats[:, tt, 1, 0], [[1, H], [0, H]]),
                    op=ALU.mult,
                )

            # o[t,h,d] = sum_g P[t,h,g] v[t,g,d]; products per tile, tree
            # fused over both tiles
            O = outp.tile([128, NT, H, HD], ATT_DT, tag="O")
            prod2 = prodp.tile([128, NT, H, HD, H], ATT_DT, tag="prod")
            for tt in range(NT):
                nc.vector.tensor_tensor(
                    out=prod2[:, tt],
                    in0=_ap(P[:, tt, 0, 0], [[H, H], [0, HD], [1, H]]),
                    in1=_ap(vtc[:, tt, 0, 0], [[0, H], [H, HD], [1, H]]),
                    op=ALU.mult,
                )
            halving_tree(
                prod2, NT * H * HD, H,
                _ap(O[:, 0, 0, 0], [[1, NT * H * HD], [1, 1]]),
            )

            # spill head-major: dst[h, t, d] = O[t, h, d]
            for tt in range(NT):
                base = aspill[n]
                for h0 in range(0, H, H // 2):
                    dst = bass.AP(
                        tensor=base.tensor,
                        offset=base.offset + h0 * T * HD + (t0 + tt * 128) * HD,
                        ap=[[HD, 128], [T * HD, H // 2], [1, HD]],
                    )
                    nc.sync.dma_start(
                        out=dst, in_=O[:, tt, h0 : h0 + H // 2, :]
                    )

        def p2_chunk(n, c, tail=False):
            # in the tail (no attention running) DVE is idle: split the
            # PSUM evacuations between DVE and ACT and keep the spill-load
            # off the ACT queue so ACT never head-of-line blocks PE
            evac = 0

            def tail_cp(**kw):
                nonlocal evac
                evac += 1
                if evac % 2:
                    nc.vector.tensor_copy(**kw)
                else:
                    nc.scalar.copy(**kw)

            t0 = c * CH
            AT = xtp.tile([128, KD, CH], ATT_DT, tag="AT")
            for tt in range(NT):
                at = xp.tile([128, D], ATT_DT, tag="a")
                dma = nc.sync.dma_start if tail else nc.scalar.dma_start
                dma(
                    out=at, in_=aspill[n, t0 + tt * 128 : t0 + tt * 128 + 128, :]
                )
                for k in range(0, KD, 2):
                    pt = psT.tile([128, 2, 128], ATT_DT, tag="tp")
                    for kk in range(2):
                        nc.tensor.transpose(
                            pt[:, kk, :],
                            at[:, (k + kk) * 128 : (k + kk + 1) * 128],
                            ident_b,
                        )
                    cp = tail_cp if tail else nc.scalar.copy
                    cp(
                        out=_ap(AT[:, k, tt * 128], [[CH, 2], [1, 128]]),
                        in_=pt,
                    )
            yT = ytp.tile([128, JP, CH], ATT_DT, tag="yT")
            for j in range(JP):
                pm = psA.tile([128, CH], F32, tag="mm")
                for k in range(KD):
                    nc.tensor.matmul(
                        pm,
                        wp_s[:, k, j * 128 : (j + 1) * 128],
                        AT[:, k, :],
                        start=(k == 0),
                        stop=(k == KD - 1),
                    )
                nc.scalar.activation(
                    out=yT[:, j, :], in_=pm, func=Ident,
                    bias=bias[:, JQ + j : JQ + j + 1], scale=1.0,
                )
            for tt in range(NT):
                yt = ytp.tile([128, D], F32, tag="y")
                for j in range(0, JP, 2):
                    pt = psT.tile([128, 2, 128], ATT_DT, tag="tp")
                    for jj in range(2):
                        nc.tensor.transpose(
                            pt[:, jj, :],
                            yT[:, j + jj, tt * 128 : (tt + 1) * 128],
                            ident_b,
                        )
                    cp = tail_cp if tail else nc.scalar.copy
                    cp(out=yt[:, j * 128 : (j + 2) * 128], in_=pt)
                r0 = n * T + t0 + tt * 128
                nc.sync.dma_start(out=y[r0 : r0 + 128, :], in_=yt)

        # interleave phase-2 (PE/ACT-heavy) one batch element behind phase-1
        # (DVE-heavy attention): p2(n, c) projects head-c rows spanning ALL
        # tokens of batch element n (the reference's h-major flatten), so it
        # can only start once every p1(n, *) chunk has spilled
        for c in range(NCH):
            p1_chunk(0, c)
        for n in range(1, NB):
            for c in range(NCH):
                p1_chunk(n, c)
                p2_chunk(n - 1, c)
        for c in range(NCH):
            p2_chunk(NB - 1, c, tail=True)

    # TRN2 allows at most one sync wait per engine instruction; split
    # multi-wait instructions through event semaphores.
    import bass_rust

    bass_rust.generate_event_semaphores(nc)
    return nc


_NC_CACHE = None
TRACE = False
LAST_RESULTS = None


def kernel(x, w_qkv, b_qkv, w_proj, b_proj):
    global _NC_CACHE, LAST_RESULTS
    if _NC_CACHE is None:
        _NC_CACHE = build_kernel()
    nc = _NC_CACHE
    x = np.ascontiguousarray(np.asarray(x, dtype=np.float32))
    w_qkv = np.ascontiguousarray(np.asarray(w_qkv, dtype=np.float32))
    b_qkv = np.ascontiguousarray(np.asarray(b_qkv, dtype=np.float32))
    w_proj = np.ascontiguousarray(np.asarray(w_proj, dtype=np.float32))
    b_proj = np.ascontiguousarray(np.asarray(b_proj, dtype=np.float32))
    in_maps = []
    for i in range(NCORES):
        in_maps.append(
            {
                "x": x[i * NB : (i + 1) * NB].reshape(NB * T, D),
                "w_qkv": w_qkv,
                "b_qkv": b_qkv,
                "w_proj": w_proj,
                "b_proj": b_proj,
            }
        )
    res = run_bass_kernel_spmd(
        nc, in_maps, core_ids=list(range(NCORES)), trace=TRACE
    )
    LAST_RESULTS = res
    out = np.empty((N, T, D), dtype=np.float32)
    for i in range(NCORES):
        out[i * NB : (i + 1) * NB] = res.results[i]["y"].reshape(NB, T, D)
    return out


if __name__ == "__main__":
    rng = np.random.default_rng(0)
    inputs = {
        "x": rng.standard_normal((N, T, D), dtype=np.float32),
        "w_qkv": rng.standard_normal((D, 3 * D), dtype=np.float32) * D**-0.5,
        "b_qkv": rng.standard_normal((3 * D,), dtype=np.float32) * 0.02,
        "w_proj": rng.standard_normal((D, D), dtype=np.float32) * D**-0.5,
        "b_proj": rng.standard_normal((D,), dtype=np.float32) * 0.02,
    }
    out = kernel(**inputs)
    print("out", out.shape, out.dtype)

